# revision 1
# baseline (speedup 1.0000x reference)
"""LCNN conv2d kernel for Trainium2 (8 NeuronCores, batch-sharded).

Math: out[b,o,h,w] = sum_d Wmat[o,d] * conv2d(x, dictionary)[b,d,h,w]
where Wmat is the scatter-add of lookup_coefficients into [O, D].

Device strategy (per core, 2 batches):
 - stage 1: conv with the D=100 dictionary as 6 accumulating matmuls per
   output tile (kernel-width pairs packed into the 128-partition
   contraction via a +1-shifted copy of x on partitions 64..127).
 - stage 2: [O=256, D=100] channel-mix matmul on the conv result.
 - all matmul inputs rounded to float32r (full PE rate, ~1e-4 rel err).
"""
import os
import sys

for _p in ("/opt/trn_rl_repo", "/root/.axon_site/_ro/trn_rl_repo"):
    if os.path.isdir(_p) and _p not in sys.path:
        sys.path.insert(0, _p)

import ml_dtypes
import numpy as np
from contextlib import ExitStack

from concourse import bacc, mybir, tile
from concourse.bass_utils import run_bass_kernel_spmd

# problem shapes (hardcoded per contract)
B, CIN, H, W = 16, 64, 96, 96
D, O = 100, 256
NCORES = 8
BPC = B // NCORES          # batches per core
PH, PW = H + 2, W + 2      # zero-padded spatial
F = BPC * PH * PW          # per-partition x extent
R = 4                      # output rows per matmul tile
NT = H // R                # h-tiles per batch
G = 4                      # h-tiles per output-DMA group
NG = NT // G
N = R * W                  # matmul free size (384)
f32 = mybir.dt.float32
f32r = mybir.dt.float32r

_NC_CACHE = {}


def _build():
    nc = bacc.Bacc(None, target_bir_lowering=False, debug=False)
    # inputs are pre-rounded to f32r on the host so loads can use the fast
    # no-cast HWDGE path (SWDGE cast DMAs measured ~120 GB/s).
    xp = nc.declare_dram_parameter("xp", [CIN, F], f32r, isOutput=False)
    wp = nc.declare_dram_parameter("wp", [128, 3 * D], f32r, isOutput=False)
    ws = nc.declare_dram_parameter("ws", [128, 3 * D], f32r, isOutput=False)
    wm = nc.declare_dram_parameter("wm", [D, O], f32r, isOutput=False)
    out = nc.declare_dram_parameter("out", [BPC, O, H, W], f32, isOutput=True)

    with tile.TileContext(nc) as tc, ExitStack() as ctx:
        sb = ctx.enter_context(tc.tile_pool(name="sb", bufs=1))
        conv1p = ctx.enter_context(tc.tile_pool(name="conv1p", bufs=3))
        stgp = ctx.enter_context(tc.tile_pool(name="stgp", bufs=2))
        pcp = ctx.enter_context(tc.tile_pool(name="pcp", bufs=2, space="PSUM"))
        pop = ctx.enter_context(tc.tile_pool(name="pop", bufs=2, space="PSUM"))

        XX = sb.tile([128, F], f32r)
        wp_s = sb.tile([128, 3 * D], f32r)
        ws_s = sb.tile([128, 3 * D], f32r)
        wm_s = sb.tile([D, O], f32r)
        nc.sync.dma_start(wp_s[:], wp[:])
        nc.sync.dma_start(ws_s[:], ws[:])
        nc.sync.dma_start(wm_s[:], wm[:])

        # x load (f32 -> f32r cast in DMA) + the +1-shifted duplicate for
        # packing two kernel-width taps into one 128-deep contraction.
        NCH = 8
        L = F // NCH
        for c in range(NCH):
            a = c * L
            nc.sync.dma_start(XX[0:CIN, a:a + L], xp[:, a:a + L])
        for c in range(NCH):
            a = c * L
            e = min(a + L, F - 1)
            nc.vector.tensor_copy(XX[64:128, a:e], XX[0:CIN, a + 1:e + 1])
        # keep the one never-paired trailing element finite: the K=128-padded
        # single-tap matmuls read it under a zero weight (NaN would poison).
        nc.vector.tensor_copy(XX[64:128, F - 1:F], XX[0:CIN, F - 1:F])

        XXv = XX.rearrange("p (b h w) -> p b h w", b=BPC, h=PH, w=PW)

        for b in range(BPC):
            for g in range(NG):
                stg = stgp.tile([128, 2 * G * N], f32, name="stg")
                for t in range(G):
                    h0 = (g * G + t) * R
                    pc = pcp.tile([D, N], f32, name="pc")
                    for kh in range(3):
                        nc.tensor.matmul(
                            pc[:], wp_s[:, kh * D:(kh + 1) * D],
                            XXv[:, b, h0 + kh:h0 + kh + R, 0:W],
                            start=(kh == 0), stop=False)
                    for kh in range(3):
                        # K padded to 128 (zero weight rows 64..127) so the
                        # accumulation group has uniform K — mixed-K groups
                        # measured ~1.5-2.5x slower per matmul.
                        nc.tensor.matmul(
                            pc[:], ws_s[:, kh * D:(kh + 1) * D],
                            XXv[:, b, h0 + kh:h0 + kh + R, 2:PW],
                            start=False, stop=(kh == 2))
                    c1 = conv1p.tile([D, N], f32r, name="c1")
                    nc.vector.tensor_copy(c1[:], pc[:])
                    po0 = pop.tile([128, N], f32, name="po0")
                    po1 = pop.tile([128, N], f32, name="po1")
                    nc.tensor.matmul(po0[:], wm_s[:, 0:128], c1[:],
                                     start=True, stop=True)
                    nc.tensor.matmul(po1[:], wm_s[:, 128:256], c1[:],
                                     start=True, stop=True)
                    nc.scalar.copy(stg[:, t * N:(t + 1) * N], po0[:])
                    nc.scalar.copy(stg[:, G * N + t * N:G * N + (t + 1) * N],
                                   po1[:])
                    if t % 2 == 1:
                        # store half the group as soon as its two tiles are
                        # evacuated — shortens the kernel tail.
                        half = t // 2
                        dst = out[b].rearrange("(u o) h w -> o u (h w)", u=2)[
                            :, :,
                            g * G * N + half * 2 * N:
                            g * G * N + (half + 1) * 2 * N]
                        src = stg.rearrange("p (u n) -> p u n", u=2)[
                            :, :, half * 2 * N:(half + 1) * 2 * N]
                        nc.gpsimd.dma_start(dst, src)

    nc.compile()
    return nc


def _get_nc():
    if "nc" not in _NC_CACHE:
        _NC_CACHE["nc"] = _build()
    return _NC_CACHE["nc"]


def _round_f32r(a):
    # round to a bf16-pair representable value (what the fp32r datapath keeps)
    hi = a.astype(ml_dtypes.bfloat16).astype(np.float32)
    lo = (a - hi).astype(ml_dtypes.bfloat16).astype(np.float32)
    return hi + lo


def _prep_inputs(x, dictionary, lookup_coefficients, lookup_indices):
    x = np.asarray(x, dtype=np.float32)
    dic = np.asarray(dictionary, dtype=np.float32)
    coeff = np.asarray(lookup_coefficients, dtype=np.float32).reshape(O, -1)
    idx = np.asarray(lookup_indices).astype(np.int64).reshape(O, -1)

    wmat = np.zeros((O, D), np.float32)
    np.add.at(wmat, (np.arange(O)[:, None], idx), coeff)
    wm = np.ascontiguousarray(wmat.T)                     # [D, O]

    dt_ = dic.transpose(1, 0, 2, 3)                       # [cin, d, kh, kw]
    wp = np.zeros((128, 3 * D), np.float32)
    wsn = np.zeros((128, 3 * D), np.float32)              # rows 64.. stay zero
    for kh in range(3):
        wp[0:64, kh * D:(kh + 1) * D] = dt_[:, :, kh, 0]
        wp[64:128, kh * D:(kh + 1) * D] = dt_[:, :, kh, 1]
        wsn[0:64, kh * D:(kh + 1) * D] = dt_[:, :, kh, 2]

    xpad = np.zeros((B, CIN, PH, PW), np.float32)
    xpad[:, :, 1:H + 1, 1:W + 1] = _round_f32r(x)
    wp = _round_f32r(wp)
    wsn = _round_f32r(wsn)
    wm = _round_f32r(wm)

    in_maps = []
    for c in range(NCORES):
        xc = xpad[c * BPC:(c + 1) * BPC].transpose(1, 0, 2, 3).reshape(CIN, F)
        in_maps.append({
            "xp": np.ascontiguousarray(xc),
            "wp": wp, "ws": wsn, "wm": wm,
        })
    return in_maps


def _run(in_maps, trace=False, **kw):
    nc = _get_nc()
    return run_bass_kernel_spmd(nc, in_maps, core_ids=list(range(NCORES)),
                                trace=trace, **kw)


def kernel(x, dictionary, lookup_coefficients, lookup_indices):
    in_maps = _prep_inputs(x, dictionary, lookup_coefficients, lookup_indices)
    res = _run(in_maps)
    outs = [res.results[c]["out"] for c in range(NCORES)]
    return np.concatenate(outs, axis=0)



# revision 3
# speedup vs baseline: 1.2249x; 1.2249x over previous
"""LCNN conv2d kernel for Trainium2 (8 NeuronCores, batch-sharded).

Math: out[b,o,h,w] = sum_d Wmat[o,d] * conv2d(x, dictionary)[b,d,h,w]
where Wmat is the scatter-add of lookup_coefficients into [O, D].

Device strategy (per core, 2 batches), all-bf16:
 - stage 1: conv with the D=100 (padded to 128) dictionary as 5 accumulating
   K=128 matmuls per output tile: 3 kw-pair matmuls on XXa=[x ; x+1col],
   1 kh-pair matmul (kh1/kh2 @ kw2) on XXc=[x+1row ; x+2row], and 1
   single-tap matmul (kh0 @ kw2, zero-padded K) on XXa.
 - stage 2: [O=256, D] channel-mix as 2 matmuls (128-wide halves) on the
   bf16 copy of the conv PSUM tile.
 - output staged to SBUF as bf16 and DMA'd at half the f32 byte cost;
   host upcasts to f32.
"""
import os
import sys

for _p in ("/opt/trn_rl_repo", "/root/.axon_site/_ro/trn_rl_repo"):
    if os.path.isdir(_p) and _p not in sys.path:
        sys.path.insert(0, _p)

import ml_dtypes
import numpy as np
from contextlib import ExitStack

from concourse import bacc, mybir, tile
from concourse.bass_utils import run_bass_kernel_spmd

# problem shapes (hardcoded per contract)
B, CIN, H, W = 16, 64, 96, 96
D, O = 100, 256
DP = 128                   # D padded to full PE width (enables FWL)
NCORES = 8
BPC = B // NCORES          # batches per core
PH, PW = H + 2, W + 2      # zero-padded spatial
F = BPC * PH * PW          # per-partition x extent
R = 4                      # output rows per matmul tile
NT = H // R                # h-tiles per batch
N = R * W                  # matmul free size (384)
NCH = 8                    # x load chunks
LAG = 2                    # stage-2 pipeline lag (tiles)
bf16 = mybir.dt.bfloat16
f32 = mybir.dt.float32

_NC_CACHE = {}


def _build():
    nc = bacc.Bacc(None, target_bir_lowering=False, debug=False)
    xp = nc.declare_dram_parameter("xp", [128, F], bf16, isOutput=False)
    xc = nc.declare_dram_parameter("xc", [128, F], bf16, isOutput=False)
    wst = nc.declare_dram_parameter("wst", [128, 5 * DP], bf16, isOutput=False)
    wm = nc.declare_dram_parameter("wm", [DP, O], bf16, isOutput=False)
    out = nc.declare_dram_parameter("out", [BPC, O, H, W], bf16, isOutput=True)

    with tile.TileContext(nc) as tc, ExitStack() as ctx:
        sb = ctx.enter_context(tc.tile_pool(name="sb", bufs=1))
        c1p = ctx.enter_context(tc.tile_pool(name="c1p", bufs=4))
        stgp = ctx.enter_context(tc.tile_pool(name="stgp", bufs=3))
        pcp = ctx.enter_context(tc.tile_pool(name="pcp", bufs=4, space="PSUM"))
        pop = ctx.enter_context(tc.tile_pool(name="pop", bufs=2, space="PSUM"))

        XXa = sb.tile([128, F], bf16)
        XXc = sb.tile([128, F], bf16)
        wst_s = sb.tile([128, 5 * DP], bf16)
        wm_s = sb.tile([DP, O], bf16)
        nc.sync.dma_start(wst_s[:], wst[:])
        nc.sync.dma_start(wm_s[:], wm[:])

        L = F // NCH
        for c in range(NCH):
            a = c * L
            nc.sync.dma_start(XXa[:, a:a + L], xp[:, a:a + L])
            nc.sync.dma_start(XXc[:, a:a + L], xc[:, a:a + L])

        XAv = XXa.rearrange("p (b h w) -> p b h w", b=BPC, h=PH, w=PW)
        XCv = XXc.rearrange("p (b h w) -> p b h w", b=BPC, h=PH, w=PW)

        state = {}

        def stage1(b, t):
            h0 = t * R
            pc = pcp.tile([DP, N], f32, name="pc")
            # kw-pairs (kw0,kw1) for each kh on XXa
            for kh in range(3):
                nc.tensor.matmul(
                    pc[:], wst_s[:, kh * DP:(kh + 1) * DP],
                    XAv[:, b, h0 + kh:h0 + kh + R, 0:W],
                    start=(kh == 0), stop=False)
            # kh-pair (kh1,kh2) @ kw2 on XXc (row-shifted banks)
            nc.tensor.matmul(
                pc[:], wst_s[:, 3 * DP:4 * DP],
                XCv[:, b, h0:h0 + R, 2:PW],
                start=False, stop=False)
            # single tap (kh0, kw2): K padded with zero weight rows 64..127
            nc.tensor.matmul(
                pc[:], wst_s[:, 4 * DP:5 * DP],
                XAv[:, b, h0:h0 + R, 2:PW],
                start=False, stop=True)
            c1 = c1p.tile([DP, N], bf16, name="c1")
            nc.vector.tensor_copy(c1[:], pc[:])
            state[(b, t)] = c1

        def stage2(b, t):
            c1 = state.pop((b, t))
            po0 = pop.tile([128, N], f32, name="po0")
            po1 = pop.tile([128, N], f32, name="po1")
            nc.tensor.matmul(po0[:], wm_s[:, 0:128], c1[:],
                             start=True, stop=True)
            nc.tensor.matmul(po1[:], wm_s[:, 128:256], c1[:],
                             start=True, stop=True)
            tt = t % 2
            if tt == 0:
                state["stg"] = stgp.tile([128, 4 * N], bf16, name="stg")
            stg = state["stg"]
            nc.scalar.copy(stg[:, tt * N:(tt + 1) * N], po0[:])
            nc.vector.tensor_copy(
                stg[:, 2 * N + tt * N:2 * N + (tt + 1) * N], po1[:])
            if tt == 1:
                # two tiles staged -> one DMA: partition o carries channels
                # {o, 128+o}, each a contiguous 768-elem (1.5 KB) run.
                pr = t // 2
                dst = out[b].rearrange("(u o) h w -> o u (h w)", u=2)[
                    :, :, pr * 2 * N:(pr + 1) * 2 * N]
                src = stg.rearrange("p (u n) -> p u n", u=2)
                nc.gpsimd.dma_start(dst, src)

        for b in range(BPC):
            for t in range(NT):
                stage1(b, t)
                if t >= LAG:
                    stage2(b, t - LAG)
            for t in range(NT - LAG, NT):
                stage2(b, t)

    nc.compile()
    return nc


def _get_nc():
    if "nc" not in _NC_CACHE:
        _NC_CACHE["nc"] = _build()
    return _NC_CACHE["nc"]


def _prep_inputs(x, dictionary, lookup_coefficients, lookup_indices):
    x = np.asarray(x, dtype=np.float32)
    dic = np.asarray(dictionary, dtype=np.float32)
    coeff = np.asarray(lookup_coefficients, dtype=np.float32).reshape(O, -1)
    idx = np.asarray(lookup_indices).astype(np.int64).reshape(O, -1)

    wmat = np.zeros((O, D), np.float32)
    np.add.at(wmat, (np.arange(O)[:, None], idx), coeff)
    wmp = np.zeros((DP, O), np.float32)
    wmp[:D] = wmat.T
    wmp = wmp.astype(ml_dtypes.bfloat16)

    # stationary slabs [128, 5*DP]: 3 kw-pairs, the (kh1,kh2)@kw2 pair,
    # and the lone (kh0,kw2) tap (upper K rows zero).
    dt_ = dic.transpose(1, 0, 2, 3)                       # [cin, d, kh, kw]
    wstk = np.zeros((128, 5 * DP), np.float32)
    for kh in range(3):
        wstk[0:64, kh * DP:kh * DP + D] = dt_[:, :, kh, 0]
        wstk[64:128, kh * DP:kh * DP + D] = dt_[:, :, kh, 1]
    wstk[0:64, 3 * DP:3 * DP + D] = dt_[:, :, 1, 2]
    wstk[64:128, 3 * DP:3 * DP + D] = dt_[:, :, 2, 2]
    wstk[0:64, 4 * DP:4 * DP + D] = dt_[:, :, 0, 2]
    wstk = wstk.astype(ml_dtypes.bfloat16)

    xpad = np.zeros((B, CIN, PH, PW), np.float32)
    xpad[:, :, 1:H + 1, 1:W + 1] = x
    xpad = xpad.astype(ml_dtypes.bfloat16)

    in_maps = []
    for c in range(NCORES):
        xf = xpad[c * BPC:(c + 1) * BPC].transpose(1, 0, 2, 3).reshape(CIN, F)
        xpk = np.zeros((128, F), ml_dtypes.bfloat16)
        xck = np.zeros((128, F), ml_dtypes.bfloat16)
        xpk[0:64] = xf
        xpk[64:128, 0:F - 1] = xf[:, 1:]
        xck[0:64, 0:F - PW] = xf[:, PW:]
        xck[64:128, 0:F - 2 * PW] = xf[:, 2 * PW:]
        in_maps.append({
            "xp": np.ascontiguousarray(xpk),
            "xc": np.ascontiguousarray(xck),
            "wst": wstk, "wm": wmp,
        })
    return in_maps


def _run(in_maps, trace=False, **kw):
    nc = _get_nc()
    return run_bass_kernel_spmd(nc, in_maps, core_ids=list(range(NCORES)),
                                trace=trace, **kw)


def kernel(x, dictionary, lookup_coefficients, lookup_indices):
    in_maps = _prep_inputs(x, dictionary, lookup_coefficients, lookup_indices)
    res = _run(in_maps)
    outs = [np.asarray(res.results[c]["out"]).astype(np.float32)
            for c in range(NCORES)]
    return np.concatenate(outs, axis=0)
